# revision 9
# baseline (speedup 1.0000x reference)
"""ChebConv (K=4) Trainium2 kernel: y = sum_k W_k @ x_k, x_{k+1} = 2 L x_k - x_{k-1}.

Strategy (graph-parallel / V-sharded over 8 NeuronCores):
  - State table x_k stored [VP=50176, 512] bf16 where col = fin*8 + b
    (all 8 batches share one SpMM since L is batch-invariant).
  - Core r owns destination rows [r*6272, (r+1)*6272) = 49 windows of 128.
    Per pass it processes only its ~100k edges; each edge dma_gathers a
    1KB bf16 source row (512 feats) from the full table (two half-tables
    so indices fit dma_gather's int16 limit).
  - Per 128-edge chunk: one-hot scaled scatter matrix S[e, r] (bf16, built
    with a single tensor_scalar) and PE accumulates S.T @ G into a
    [128, 512] f32 PSUM bank; x_{k+1} = psum - x_{k-1} (vals pre-doubled
    on host for k >= 2).
  - After passes 1 and 2 an AllGather (bf16, 6.4MB per core) rebuilds the
    full table for the next pass's gathers.
  - Projection is fused per window: per batch, PE-transpose of the fresh
    x_k tile, then matmul against W_k (bf16); y accumulates across passes
    in a persistent f32 SBUF tile and is written out once at the end.
All preprocessing (transpose/sort/pad/index layout) happens on host.
"""

import sys

for _p in ("/opt/trn_rl_repo",):
    if _p not in sys.path:
        sys.path.insert(0, _p)

import numpy as np
import ml_dtypes

import concourse.bass as bass
import concourse.bacc as bacc
import concourse.mybir as mybir
from concourse import tile
from concourse.bass_utils import run_bass_kernel_spmd

B, FIN, FOUT, V, K, NNZ = 8, 64, 64, 50000, 4, 800000
F = 64
C = FIN * B              # 512 table columns, col = fin*8 + b
VP = 50176               # padded nodes: 392 windows of 128
HALF = VP // 2           # 25088 (< 2**15 so gather idx fits int16)
NCORE = 8
SLICE = VP // NCORE      # 6272 rows per core
NWC = SLICE // 128       # 49 windows per core
NWIN = VP // 128         # 392


def _apply_drain_patch():
    """This walrus build rejects >1 sync waits on the Tile kernel-tail Drain
    (NO_STRUCT codegen path). Emit explicit per-sem EVSEM waits on SP first so
    the drain itself needs none."""
    import bass_rust

    def _patched(self, tick_clock, wait_clock):
        gc = list(tick_clock.global_clock)
        sems = self.sems.allocated()
        for proc, sem in sems.items():
            tick = gc[proc] if proc < len(gc) else 0
            if tick <= 0:
                continue
            name = getattr(sem, "name", "") or ""
            mult = 16 if "DMA" in name else 1
            self.nc.sync.wait_ge(sem, tick * mult)
        self.nc.sync.drain()
        self.nc.all_engine_barrier()
        assert self.sems is not None
        popped = self.nc._tile_sem_poison_stack.pop()
        assert popped is self._sem_poison
        self.nc.clear_and_free_semaphores(list(self.sems.allocated().values()))
        self.nc.all_engine_barrier()

    tile.TileContext._drain_and_barrier = _patched


_apply_drain_patch()


# --------------------------------------------------------------------------
# Host-side edge preprocessing
# --------------------------------------------------------------------------

def preprocess_edges(rows, cols, vals):
    """Sort edges into a uniform [window, half, chunk, lane] grid.

    Returns eidx [NWIN*128, (CWL+CWH)*8] i16, rv1/rv2 [NWIN*128, 2*CW] f32
    and the per-half chunk counts (CWL, CWH)."""
    rows = np.asarray(rows, np.int64)
    cols = np.asarray(cols, np.int64)
    vals = np.asarray(vals, np.float32)

    w = rows // 128
    rloc = (rows % 128).astype(np.float32)
    ishi = (cols >= HALF).astype(np.int64)
    key = w * 2 + ishi
    order = np.argsort(key, kind="stable")
    cnt = np.bincount(key[order], minlength=NWIN * 2).reshape(NWIN, 2)
    CWL = max(1, int(-(-cnt[:, 0].max() // 128)))
    CWH = max(1, int(-(-cnt[:, 1].max() // 128)))
    CW = CWL + CWH

    idx16 = np.zeros((NWIN, 2, max(CWL, CWH) * 128), np.int16)
    rl = np.zeros((NWIN, 2, max(CWL, CWH) * 128), np.float32)
    vv = np.zeros((NWIN, 2, max(CWL, CWH) * 128), np.float32)
    offs = np.concatenate([[0], np.cumsum(cnt.reshape(-1))])
    scol = cols[order]
    srl = rloc[order]
    sval = vals[order]
    for wi in range(NWIN):
        for h in (0, 1):
            n = cnt[wi, h]
            if not n:
                continue
            o = offs[wi * 2 + h]
            idx16[wi, h, :n] = (scol[o:o + n] - h * HALF).astype(np.int16)
            rl[wi, h, :n] = srl[o:o + n]
            vv[wi, h, :n] = sval[o:o + n]

    def wrap16(a):
        # flat desc list [N] -> [128, N//16]: desc d at (part d%16, col d//16),
        # replicated x8 across partition groups.
        n = a.shape[-1]
        b = a.reshape(*a.shape[:-1], n // 16, 16)
        b = np.swapaxes(b, -1, -2)
        return np.tile(b, (1, 8, 1))

    lo = wrap16(idx16[:, 0, :CWL * 128])        # [NWIN, 128, CWL*8]
    hi = wrap16(idx16[:, 1, :CWH * 128])        # [NWIN, 128, CWH*8]
    eidx = np.concatenate([lo, hi], axis=2).reshape(NWIN * 128, CW * 8)
    eidx = np.ascontiguousarray(eidx, np.int16)

    # rv: [NWIN*128, 2*CW]; col c<CW: rowloc of chunk c, col CW+c: val.
    # chunk order: CWL lo chunks then CWH hi chunks.
    def chunkize(a, ncw, h):
        return a[:, h, :ncw * 128].reshape(NWIN, ncw, 128).transpose(0, 2, 1)

    rl_c = np.concatenate([chunkize(rl, CWL, 0), chunkize(rl, CWH, 1)], axis=2)
    vv_c = np.concatenate([chunkize(vv, CWL, 0), chunkize(vv, CWH, 1)], axis=2)
    rv1 = np.concatenate([rl_c, vv_c], axis=2).reshape(NWIN * 128, 2 * CW)
    rv2 = np.concatenate([rl_c, vv_c * 2.0], axis=2).reshape(NWIN * 128, 2 * CW)
    return dict(eidx=eidx, rv1=np.ascontiguousarray(rv1, np.float32),
                rv2=np.ascontiguousarray(rv2, np.float32), CWL=CWL, CWH=CWH)


# --------------------------------------------------------------------------
# Device program
# --------------------------------------------------------------------------

_ABL = dict(no_proj=False, no_ag=False)


def build_program(CWL, CWH):
    CW = CWL + CWH
    no_proj = _ABL["no_proj"]
    no_ag = _ABL["no_ag"]
    f32 = mybir.dt.float32
    bf16 = mybir.dt.bfloat16
    i16 = mybir.dt.int16
    ie = mybir.AluOpType.is_equal
    mu = mybir.AluOpType.mult
    sb = mybir.AluOpType.subtract
    ad = mybir.AluOpType.add

    NQ = 4
    nc = bacc.Bacc("TRN2", target_bir_lowering=False, num_swdge_queues=NQ,
                   num_devices=NCORE)
    xg0 = nc.dram_tensor("xg0", [VP, C], bf16, kind="ExternalInput")
    x0loc = nc.dram_tensor("x0loc", [SLICE, C], bf16, kind="ExternalInput")
    eidx = nc.dram_tensor("eidx", [SLICE, CW * 8], i16, kind="ExternalInput")
    rv1 = nc.dram_tensor("rv1", [SLICE, 2 * CW], f32, kind="ExternalInput")
    rv2 = nc.dram_tensor("rv2", [SLICE, 2 * CW], f32, kind="ExternalInput")
    wk = nc.dram_tensor("wk", [F, K * F], bf16, kind="ExternalInput")
    iden = nc.dram_tensor("iden", [128, 128], bf16, kind="ExternalInput")
    iota = nc.dram_tensor("iota", [128, 128], f32, kind="ExternalInput")
    yout = nc.dram_tensor("yout", [SLICE, C], f32, kind="ExternalOutput")

    ag1 = nc.dram_tensor("ag1", [SLICE, C], bf16)
    ag2 = nc.dram_tensor("ag2", [SLICE, C], bf16)
    xg1 = nc.dram_tensor("xg1", [VP, C], bf16, addr_space="Shared")
    xg2 = nc.dram_tensor("xg2", [VP, C], bf16, addr_space="Shared")

    from concourse import library_config

    with tile.TileContext(nc) as tc:
        nc.gpsimd.load_library(library_config.mlp)
        from contextlib import ExitStack
        with ExitStack() as stk:
            cpool = stk.enter_context(tc.tile_pool(name="const", bufs=1))
            dpool = stk.enter_context(tc.tile_pool(name="data", bufs=3))
            gpool = stk.enter_context(tc.tile_pool(name="gath", bufs=3))
            spool = stk.enter_context(tc.tile_pool(name="smat", bufs=8))
            xpool = stk.enter_context(tc.tile_pool(name="xtil", bufs=4))
            tpool = stk.enter_context(tc.tile_pool(name="xt", bufs=6))
            psx_p = stk.enter_context(tc.tile_pool(name="psx", bufs=2, space="PSUM"))
            pst_p = stk.enter_context(tc.tile_pool(name="pst", bufs=3, space="PSUM"))
            psy_p = stk.enter_context(tc.tile_pool(name="psy", bufs=2, space="PSUM"))

            iota_t = cpool.tile([128, 128], f32)
            nc.sync.dma_start(iota_t[:], iota[:])
            iden_t = cpool.tile([128, 128], bf16)
            nc.sync.dma_start(iden_t[:], iden[:])
            wk_t = cpool.tile([F, K * F], bf16)
            nc.sync.dma_start(wk_t[:], wk[:])
            y_sb = cpool.tile([128, NWC, C], f32)

            qn = [0]

            def window_body(p, g):
                """pass p in {1,2,3}, window g in [0, NWC)."""
                src = {1: xg0, 2: xg1, 3: xg2}[p]
                prev = {1: None, 2: x0loc, 3: ag1}[p]
                agdst = {1: ag1, 2: ag2, 3: None}[p]
                rv_d = rv1 if p == 1 else rv2
                r0 = g * 128

                idx_t = dpool.tile([128, CW * 8], i16, tag="idx")
                nc.sync.dma_start(idx_t[:], eidx[bass.ds(r0, 128), :])
                rv_t = dpool.tile([128, 2 * CW], f32, tag="rv")
                nc.scalar.dma_start(rv_t[:], rv_d[bass.ds(r0, 128), :])

                g_lo = gpool.tile([128, CWL, C], bf16, tag="glo")
                g_hi = gpool.tile([128, CWH, C], bf16, tag="ghi")
                # SWDGE ring caps one gather at 1024 descriptors; emit
                # <=8-chunk pieces round-robined over the queues.
                for half, gt, ncw, coff in ((0, g_lo, CWL, 0),
                                            (1, g_hi, CWH, CWL * 8)):
                    sap = src[bass.ds(half * HALF, HALF), :]
                    for c0 in range(0, ncw, 8):
                        cn = min(8, ncw - c0)
                        nc.gpsimd.dma_gather(
                            gt[:, c0:c0 + cn, :], sap,
                            idx_t[:, coff + c0 * 8:coff + (c0 + cn) * 8],
                            cn * 128, cn * 128, C, queue_num=qn[0] % NQ)
                        qn[0] += 1

                if prev is not None:
                    prev_t = dpool.tile([128, C], bf16, tag="prev")
                    nc.scalar.dma_start(prev_t[:], prev[bass.ds(r0, 128), :])
                if p == 1:
                    x0p_t = dpool.tile([128, F, B], bf16, tag="x0p")
                    nc.scalar.dma_start(
                        x0p_t[:], x0loc[bass.ds(r0, 128), :])

                psx = psx_p.tile([128, C], f32, tag="psx")
                for c in range(CW):
                    s_t = spool.tile([128, 128], bf16, tag="s")
                    nc.vector.tensor_scalar(
                        s_t[:], iota_t[:], rv_t[:, c:c + 1],
                        rv_t[:, CW + c:CW + c + 1], ie, mu)
                    rhs = g_lo[:, c, :] if c < CWL else g_hi[:, c - CWL, :]
                    nc.tensor.matmul(psx[:], s_t[:], rhs,
                                     start=(c == 0), stop=(c == CW - 1))

                xnb = xpool.tile([128, F, B], bf16, tag="xnb")
                psx3 = psx[:].rearrange("p (a b) -> p a b", b=B)
                if prev is None:
                    nc.vector.tensor_copy(xnb[:], psx3)
                else:
                    pvf = xpool.tile([128, C], f32, tag="pvf")
                    nc.vector.tensor_copy(pvf[:], prev_t[:])
                    pvf3 = pvf[:].rearrange("p (a b) -> p a b", b=B)
                    nc.vector.tensor_tensor(xnb[:], psx3, pvf3, sb)
                if agdst is not None:
                    nc.sync.dma_start(agdst[bass.ds(r0, 128), :], xnb[:])

                if no_proj:
                    if p == 3:
                        ys_t = xpool.tile([128, C], f32, tag="ys")
                        nc.vector.tensor_copy(ys_t[:], psx[:])
                        nc.sync.dma_start(yout[bass.ds(r0, 128), :], ys_t[:])
                    return

                # projection: psy[:, b, :] += xnb_b.T @ W_p (+ x0_b.T @ W_0)
                psy = psy_p.tile([128, B, F], f32, tag="psy")
                for b in range(B):
                    if p == 1:
                        pst0 = pst_p.tile([F, 128], bf16, tag="pst")
                        nc.tensor.transpose(pst0[:], x0p_t[:, :, b], iden_t[:])
                        xt0 = tpool.tile([F, 128], bf16, tag="xt")
                        (nc.scalar.copy if b % 2 else nc.vector.tensor_copy)(
                            xt0[:], pst0[:])
                        nc.tensor.matmul(psy[:, b, :], xt0[:], wk_t[:, 0:F],
                                         start=True, stop=False)
                    pst = pst_p.tile([F, 128], bf16, tag="pst")
                    nc.tensor.transpose(pst[:], xnb[:, :, b], iden_t[:])
                    xt = tpool.tile([F, 128], bf16, tag="xt")
                    (nc.scalar.copy if b % 2 else nc.vector.tensor_copy)(
                        xt[:], pst[:])
                    nc.tensor.matmul(psy[:, b, :], xt[:],
                                     wk_t[:, p * F:(p + 1) * F],
                                     start=(p != 1), stop=True)

                ysl = y_sb[:, g, :]
                psyf = psy[:].rearrange("p a b -> p (a b)")
                if p == 1:
                    nc.vector.tensor_copy(ysl, psyf)
                elif p == 2:
                    nc.vector.tensor_tensor(ysl, ysl, psyf, ad)
                else:
                    ys_t = xpool.tile([128, C], f32, tag="ys")
                    nc.vector.tensor_tensor(ys_t[:], ysl, psyf, ad)
                    nc.sync.dma_start(yout[bass.ds(r0, 128), :], ys_t[:])

            for p in (1, 2, 3):
                for g in range(NWC):
                    window_body(p, g)
                if p < 3 and not no_ag:
                    agt = {1: ag1, 2: ag2}[p]
                    xgt = {1: xg1, 2: xg2}[p]
                    nc.gpsimd.collective_compute(
                        "AllGather", mybir.AluOpType.bypass,
                        replica_groups=[list(range(NCORE))],
                        ins=[agt[:]],
                        outs=[xgt[:]],
                    )

    nc.compile()
    return nc


# --------------------------------------------------------------------------
# Full-size entry point
# --------------------------------------------------------------------------

def _host_inputs(x, L_vals, W, L_rows, L_cols):
    x = np.asarray(x, np.float32)
    W = np.asarray(W, np.float32)
    pre = preprocess_edges(np.asarray(L_rows), np.asarray(L_cols),
                           np.asarray(L_vals, np.float32))
    x0tab = np.zeros((VP, C), np.float32)
    x0tab[:V] = np.transpose(x, (2, 1, 0)).reshape(V, C)
    x0tab = x0tab.astype(ml_dtypes.bfloat16)

    # wk[fin, k*F+fout] = W[fout, fin*K+k]
    wk_host = np.ascontiguousarray(
        W.reshape(FOUT, FIN, K).transpose(1, 2, 0).reshape(FIN, K * FOUT)
    ).astype(ml_dtypes.bfloat16)
    iden = np.eye(128, dtype=ml_dtypes.bfloat16)
    iota = np.tile(np.arange(128, dtype=np.float32), (128, 1))

    per_core = []
    for r in range(NCORE):
        sl = slice(r * SLICE, (r + 1) * SLICE)
        per_core.append({
            "xg0": x0tab,
            "x0loc": np.ascontiguousarray(x0tab[sl]),
            "eidx": np.ascontiguousarray(pre["eidx"][sl]),
            "rv1": np.ascontiguousarray(pre["rv1"][sl]),
            "rv2": np.ascontiguousarray(pre["rv2"][sl]),
            "wk": wk_host, "iden": iden, "iota": iota,
        })
    return per_core, pre


_CACHED = {}


def _get_program(pre):
    key = (pre["CWL"], pre["CWH"], tuple(sorted(_ABL.items())))
    if key not in _CACHED:
        _CACHED[key] = build_program(pre["CWL"], pre["CWH"])
    return _CACHED[key]


def _assemble(y_all):
    # y_all: [VP, C] f32 with col = b*F + fout
    out = np.empty((B, FOUT, V), np.float32)
    for b in range(B):
        out[b] = y_all[:V, b * F:(b + 1) * F].T
    return out


def kernel(x, L_vals, W, L_rows, L_cols):
    per_core, pre = _host_inputs(x, L_vals, W, L_rows, L_cols)
    nc = _get_program(pre)
    res = run_bass_kernel_spmd(nc, per_core, list(range(NCORE)))
    y_all = np.concatenate(
        [np.asarray(res.results[r]["yout"]) for r in range(NCORE)], axis=0)
    return _assemble(y_all)


def bench(x, L_vals, W, L_rows, L_cols, reps=5):
    """Steady-state wall timing of the on-device executable (inputs resident;
    only the donated zero output buffers are re-staged outside the timed span)."""
    import time

    import jax
    from jax.sharding import Mesh, PartitionSpec
    from jax.experimental.shard_map import shard_map
    from concourse import bass2jax

    per_core, pre = _host_inputs(x, L_vals, W, L_rows, L_cols)
    nc = _get_program(pre)
    bass2jax.install_neuronx_cc_hook()

    import concourse.mybir as _mb
    in_names, out_names, out_avals, zero_outs = [], [], [], []
    for alloc in nc.m.functions[0].allocations:
        if not isinstance(alloc, _mb.MemoryLocationSet):
            continue
        name = alloc.memorylocations[0].name
        pid_name = nc.partition_id_tensor.name if nc.partition_id_tensor else None
        if alloc.kind == "ExternalInput":
            if name != pid_name:
                in_names.append(name)
        elif alloc.kind == "ExternalOutput":
            out_names.append(name)
            shape = tuple(alloc.tensor_shape)
            dtype = _mb.dt.np(alloc.dtype)
            out_avals.append(jax.core.ShapedArray(shape, dtype))
            zero_outs.append(np.zeros(shape, dtype))
    n_params = len(in_names)
    n_outs = len(out_avals)
    all_names = in_names + out_names
    if nc.partition_id_tensor:
        all_names.append(nc.partition_id_tensor.name)

    def _body(*args):
        operands = list(args)
        if nc.partition_id_tensor:
            operands.append(bass2jax.partition_id_tensor())
        outs = bass2jax._bass_exec_p.bind(
            *operands, out_avals=tuple(out_avals), in_names=tuple(all_names),
            out_names=tuple(out_names), lowering_input_output_aliases=(),
            sim_require_finite=True, sim_require_nnan=True, nc=nc)
        return tuple(outs)

    devices = jax.devices()[:NCORE]
    mesh = Mesh(np.asarray(devices), ("core",))
    donate = tuple(range(n_params, n_params + n_outs))
    sharded = jax.jit(
        shard_map(_body, mesh=mesh,
                  in_specs=(PartitionSpec("core"),) * (n_params + n_outs),
                  out_specs=(PartitionSpec("core"),) * n_outs, check_rep=False),
        donate_argnums=donate, keep_unused=True)
    concat_in = [np.concatenate([np.asarray(per_core[c][nm]) for c in range(NCORE)], axis=0)
                 for nm in in_names]
    sh_in = jax.sharding.NamedSharding(mesh, PartitionSpec("core"))
    in_dev = [jax.device_put(a, sh_in) for a in concat_in]
    times = []
    outs = None
    for _ in range(reps):
        zs = [jax.device_put(np.zeros((NCORE * z.shape[0], *z.shape[1:]), z.dtype), sh_in)
              for z in zero_outs]
        jax.block_until_ready(zs)
        t0 = time.perf_counter()
        outs = sharded(*in_dev, *zs)
        jax.block_until_ready(outs)
        times.append(time.perf_counter() - t0)

    # Marginal per-exec device time: back-to-back async dispatches pipeline
    # through the tunnel, so (t_N - t_1)/(N-1) isolates device execution.
    chain_times = {}
    for n in (1, 33):
        best = None
        for _ in range(4):
            zsl = [[jax.device_put(
                np.zeros((NCORE * z.shape[0], *z.shape[1:]), z.dtype), sh_in)
                for z in zero_outs] for _ in range(n)]
            jax.block_until_ready(zsl)
            t0 = time.perf_counter()
            outs_l = [sharded(*in_dev, *zs) for zs in zsl]
            jax.block_until_ready(outs_l)
            dt = time.perf_counter() - t0
            best = dt if best is None else min(best, dt)
        chain_times[n] = best
    per_exec = (chain_times[33] - chain_times[1]) / 32.0
    bench.chain_times = chain_times
    bench.per_exec_s = per_exec
    y_all = np.asarray(outs[out_names.index("yout")]).reshape(VP, C)
    return _assemble(y_all), times
